# revision 5
# baseline (speedup 1.0000x reference)
"""Causal self-attention (B=2, T=2048, C=1024, H=16) on 8 trn2 NeuronCores.

Sharding: core c handles batch b=c//4 and head group g=c%4 (4 heads each).
Data parallel on B, tensor parallel on H; W_attn/W_proj sliced per head
group; host sums the 4 tensor-parallel partial projection outputs per batch.

v2 layout (vs v1):
  - S^T matmuls for the two heads of a pair are emitted interleaved; their
    lhsT/rhs live at partitions 0-63 / 64-127, so bass auto-derives PE
    tile_position (0,0)/(64,0) and the K=64 matmuls run CONCURRENTLY on the
    two row-halves of the PE array (~2x on the S phase).
  - causal diagonal block is masked multiplicatively on DVE after exp
    (upper-triangular 0/1 mask) instead of a -8192 PE matmul accumulate;
    the full-array ident matmul would have serialized the row-tiled pairs.
  - psum->sbuf extracts (qkT bias add, proj copy) moved from ACT to DVE;
    ACT does exp only.
  - qkT tiles for heads 2,3 and the projection of query-pair 1 are emitted
    as filler units inside the attention loops of heads 0,1 (PE work that
    hides under the ACT-bound exp stream); their psum shares the "s" tag
    ring (2 x [128,1024] slots).
  - projection partials are written fp16 (halves out DMA + DVE copy);
    host accumulates in fp32.
PSUM budget: s-tag 2x[128,1024] (8KB/part) + y 4x[128,512] (8KB) = 16KB.
"""

import os
import numpy as np
import ml_dtypes

import concourse.bacc as bacc
import concourse.mybir as mybir
import concourse.tile as tile
from concourse.bass_utils import run_bass_kernel_spmd
from concourse.masks import make_identity, make_lower_triangular

B, T, C, H = 2, 2048, 1024, 16
D = C // H          # 64
HPC = H // 4        # 4 heads per core
QK = 2 * HPC * D    # 512 rows of qkT (q then k)
V = HPC * D         # 256 v columns
F32 = mybir.dt.float32
F16 = mybir.dt.float16
BF16 = mybir.dt.bfloat16
PAIR = 1024         # queries per attention pass (2 psum banks)
AF = mybir.ActivationFunctionType

_cache = {}


def _build():
    nc = bacc.Bacc("TRN2", target_bir_lowering=False, debug=False, num_devices=8)
    xT = nc.dram_tensor("xT", [C, T], BF16, kind="ExternalInput").ap()
    w_qk = nc.dram_tensor("w_qk", [C, QK], BF16, kind="ExternalInput").ap()
    b_qk = nc.dram_tensor("b_qk", [QK, 1], F32, kind="ExternalInput").ap()
    w_v = nc.dram_tensor("w_v", [C, V], BF16, kind="ExternalInput").ap()
    b_v = nc.dram_tensor("b_v", [1, V], F32, kind="ExternalInput").ap()
    w_pr = nc.dram_tensor("w_pr", [V, C], BF16, kind="ExternalInput").ap()
    out = nc.dram_tensor("out", [T, C], F16, kind="ExternalOutput").ap()

    NC_ = C // 128  # 8 c-tiles

    with tile.TileContext(nc) as tc:
        with (
            tc.tile_pool(name="const", bufs=1) as cpool,
            tc.tile_pool(name="xt", bufs=1) as xpool,
            tc.tile_pool(name="wqk", bufs=1) as wqkpool,
            tc.tile_pool(name="wv", bufs=1) as wvpool,
            tc.tile_pool(name="wpr", bufs=1) as wprpool,
            tc.tile_pool(name="qk", bufs=1) as qkpool,
            tc.tile_pool(name="vaug", bufs=1) as vpool,
            tc.tile_pool(name="att_sb", bufs=1) as apool,
            tc.tile_pool(name="osb", bufs=1) as opool,
        ):
            # ---- constants ----
            ident = cpool.tile([128, 128], BF16, name="ident")
            make_identity(nc, ident[:])
            trineg = cpool.tile([128, 128], BF16, name="trineg")
            make_lower_triangular(nc, trineg[:], val=-8192.0, diag=False)
            ones4 = cpool.tile([128, HPC], F32, name="ones4")
            nc.gpsimd.memset(ones4[:], 1.0)

            # ---- input DMA: wqk[c] then xt[c] halves, so compute can start
            # after the first pair lands ----
            xt, wqk_t, bqk_t = [], [], []
            bv_full = None
            for c in range(NC_):
                w = wqkpool.tile([128, QK], BF16, name=f"wqk{c}")
                if c == 0:
                    nc.sync.dma_start(w[:, 0:128], w_qk[0:128, 0:128])
                    nc.sync.dma_start(w[:, 128:QK], w_qk[0:128, 128:QK])
                else:
                    nc.sync.dma_start(w[:], w_qk[c * 128:(c + 1) * 128, :])
                wqk_t.append(w)
                t = xpool.tile([128, T], BF16, name=f"xt{c}")
                if c < 2:
                    for q in range(4):
                        nc.sync.dma_start(
                            t[:, q * 512:(q + 1) * 512],
                            xT[c * 128:(c + 1) * 128, q * 512:(q + 1) * 512])
                else:
                    nc.sync.dma_start(t[:, 0:1024],
                                      xT[c * 128:(c + 1) * 128, 0:1024])
                    nc.sync.dma_start(t[:, 1024:T],
                                      xT[c * 128:(c + 1) * 128, 1024:T])
                xt.append(t)
                if c == 0:
                    for j in range(QK // 128):
                        bt = cpool.tile([128, 1], F32, name=f"bqk{j}")
                        nc.sync.dma_start(bt[:], b_qk[j * 128:(j + 1) * 128, :])
                        bqk_t.append(bt)
                    bv_row = cpool.tile([1, V], F32, name="bv_row")
                    nc.sync.dma_start(bv_row[:], b_v[:])
                    bv_full = cpool.tile([128, V], F32, name="bv_full")
                    nc.gpsimd.partition_broadcast(bv_full[:], bv_row[:])
            wv_t = []
            for c in range(NC_):
                t = wvpool.tile([128, V], BF16, name=f"wv{c}")
                nc.sync.dma_start(t[:], w_v[c * 128:(c + 1) * 128, :])
                wv_t.append(t)
            wpr_t = []
            for k in range(V // 128):
                t = wprpool.tile([128, C], BF16, name=f"wpr{k}")
                nc.sync.dma_start(t[:], w_pr[k * 128:(k + 1) * 128, :])
                wpr_t.append(t)

            qk_t = [qkpool.tile([128, T], BF16, name=f"qk{j}")
                    for j in range(QK // 128)]
            # per-head stationary layout: col 0 = ones (softmax denominator
            # -> psum row 0, where reciprocal_approx_fast requires its
            # input), cols 64..127 = v rows (y -> psum rows 64..127).
            v_t = [vpool.tile([128, HPC, 128], BF16, name=f"v{t}")
                   for t in range(T // 128)]

            # ================ phase A: qkT j-tiles 0,2 (heads 0,1) ========
            with tc.tile_pool(name="ps12", bufs=1, space="PSUM") as ps12:
                ps_grp = {}
                for j in (0, 2):
                    for tch in range(T // 512):
                        ps_grp[j, tch] = ps12.tile(
                            [128, 512], F32, name="qk_ps", tag="qk_ps", bufs=8)
                for c in range(NC_):
                    for j in (0, 2):
                        for tch in range(T // 512):
                            nc.tensor.matmul(
                                ps_grp[j, tch][:],
                                wqk_t[c][:, j * 128:(j + 1) * 128],
                                xt[c][:, tch * 512:(tch + 1) * 512],
                                start=(c == 0), stop=(c == NC_ - 1))
                for j in (0, 2):
                    for tch in range(T // 512):
                        nc.vector.tensor_scalar_add(
                            qk_t[j][:, tch * 512:(tch + 1) * 512],
                            ps_grp[j, tch][:], bqk_t[j][:])

                # ============ phase B: v tiles 0..3 (chunk 0's keys);
                # tiles 4..15 are emitted as fillers inside attention ======
                for tt in range(4):
                    ps = ps12.tile([128, V], F32, name="v_ps",
                                   tag="qk_ps", bufs=8)
                    for c in range(NC_):
                        nc.tensor.matmul(
                            ps[:],
                            xt[c][:, tt * 128:(tt + 1) * 128],
                            wv_t[c][:],
                            start=(c == 0), stop=(c == NC_ - 1))
                    nc.vector.tensor_add(
                        v_t[tt][:, :, 64:64 + D],
                        ps[:].rearrange("p (h d) -> p h d", h=HPC),
                        bv_full[:].rearrange("p (h d) -> p h d", h=HPC))
                    nc.vector.tensor_copy(
                        v_t[tt][:, :, 0:1],
                        ones4[:].rearrange("p (h o) -> p h o", o=1))

            # ================= attention + projection =================
            # 512-query chunks, descending. Per chunk and head-pair hp the
            # two heads' S^T go into ONE merged psum tile [128, 2, 512]
            # (heads at array rows 0-63 / 64-127 -> row-tiled CONCURRENT
            # K=64 matmuls, both ready the moment one exp frees the slot).
            # One exp instruction covers both heads. PSUM: s-tag 2x2 banks
            # (double-buffered; also serves qk13/proj filler units) +
            # y-tag 4x1 banks.
            with tc.tile_pool(name="ps34", bufs=1, space="PSUM") as ps34:
                yn_t = [apool.tile([128, T], BF16, name=f"yn{k}")
                        for k in range(2)]

                def v_units():
                    # v tiles 4..15, one per unit, on the fill tag
                    for tt in range(4, T // 128):
                        ps = ps34.tile([128, 512], F32, name="v_ps2",
                                       tag="fill", bufs=2)
                        for cc in range(NC_):
                            nc.tensor.matmul(
                                ps[:, 0:V],
                                xt[cc][:, tt * 128:(tt + 1) * 128],
                                wv_t[cc][:],
                                start=(cc == 0), stop=(cc == NC_ - 1))
                        nc.vector.tensor_add(
                            v_t[tt][:, :, 64:64 + D],
                            ps[:, 0:V].rearrange("p (h d) -> p h d", h=HPC),
                            bv_full[:].rearrange("p (h d) -> p h d", h=HPC))
                        nc.vector.tensor_copy(
                            v_t[tt][:, :, 0:1],
                            ones4[:].rearrange("p (h o) -> p h o", o=1))
                        yield

                def qk13_units():
                    # qkT j-tiles 1,3 (heads 2,3), one psum tile per unit
                    for j in (1, 3):
                        for tch in range(T // 512):
                            ps = ps34.tile([128, 512], F32, name="qkx",
                                           tag="fill", bufs=2)
                            for c in range(NC_):
                                nc.tensor.matmul(
                                    ps[:],
                                    wqk_t[c][:, j * 128:(j + 1) * 128],
                                    xt[c][:, tch * 512:(tch + 1) * 512],
                                    start=(c == 0), stop=(c == NC_ - 1))
                            nc.vector.tensor_scalar_add(
                                qk_t[j][:, tch * 512:(tch + 1) * 512],
                                ps[:], bqk_t[j][:])
                            yield

                def proj_units(ci):
                    # projection for query chunk ci (needs yn of both hps)
                    i0c = ci * 512
                    for tt in range(4):
                        r0 = i0c + tt * 128
                        osb_t = opool.tile([128, C], F16, name="osb",
                                           tag="osb", bufs=3)
                        for cc in range(C // 512):
                            o_ps = ps34.tile([128, 512], F32, name="o_ps",
                                             tag="fill", bufs=2)
                            for k in range(2):
                                nc.tensor.matmul(
                                    o_ps[:],
                                    yn_t[k][:, r0:r0 + 128],
                                    wpr_t[k][:, cc * 512:(cc + 1) * 512],
                                    start=(k == 0), stop=(k == 1))
                            nc.vector.tensor_copy(
                                osb_t[:, cc * 512:(cc + 1) * 512],
                                o_ps[:])
                            yield
                        nc.sync.dma_start(out[r0:r0 + 128, :], osb_t[:])

                def att_chunk(hp, ci, extra, every):
                    # heads (2hp, 2hp+1), queries [512ci, 512ci+512)
                    qtile = qk_t[hp]
                    ktile = qk_t[2 + hp]
                    i0c = ci * 512
                    njt = (i0c + 512) // 128
                    y_ps = {}
                    for h2 in (0, 1):
                        y_ps[h2] = ps34.tile([128, 512], F32, name="y_ps",
                                             tag="y_ps", bufs=2)
                    for jt in range(njt):
                        if extra is not None and jt % every == 0:
                            next(extra, None)
                        j0 = jt * 128
                        lo = max(0, j0 - i0c)
                        diag = j0 >= i0c
                        s_t = ps34.tile([128, 2, 512], F32, name="s_ps",
                                        tag="s_ps", bufs=2)
                        for h2 in (0, 1):
                            qrow = h2 * D
                            nc.tensor.matmul(
                                s_t[:, h2, lo:512],
                                ktile[qrow:qrow + D, j0:j0 + 128],
                                qtile[qrow:qrow + D, i0c + lo:i0c + 512],
                                start=True, stop=not diag)
                        if diag:
                            # accumulate -8192 into the mixed diagonal block
                            # (ident.T @ trineg == trineg) so exp() yields
                            # exact zeros there; stays on the PE, no DVE hop
                            # in the exp->PV chain.
                            for h2 in (0, 1):
                                nc.tensor.matmul(
                                    s_t[:, h2, lo:lo + 128],
                                    ident[:], trineg[:],
                                    start=False, stop=True)
                        pT = apool.tile([128, 2, 512], BF16, name="pT",
                                        tag="pT", bufs=6)
                        nc.scalar.activation(
                            pT[:, :, lo:512], s_t[:, :, lo:512],
                            AF.Exp, scale=float(1.0 / np.sqrt(D)))
                        for h2 in (0, 1):
                            nc.tensor.matmul(
                                y_ps[h2][:, lo:512],
                                v_t[jt][:, 2 * hp + h2, :],
                                pT[:, h2, lo:512],
                                start=(jt == 0), stop=(jt == njt - 1))
                    # normalize: rows 64..127 divided by row 0 (l sums)
                    for h2 in (0, 1):
                        qrow = h2 * D
                        rec = apool.tile([1, 512], F32, name="rec",
                                         tag="rec", bufs=4)
                        nc.vector.reciprocal_approx_fast(
                            rec[:], y_ps[h2][0:1, :])
                        rb = apool.tile([D, 512], F32, name="rb",
                                        tag="rb", bufs=4)
                        nc.gpsimd.partition_broadcast(rb[:], rec[:])
                        nc.vector.tensor_mul(
                            yn_t[hp][qrow:qrow + D, i0c:i0c + 512],
                            y_ps[h2][64:64 + D, :], rb[:])

                # hp-major, ascending chunks. Fillers keep the PE dense
                # under the ACT-bound exp stream:
                #   hp0: c0 hosts v[4..7], c1 hosts v[8..15], c2 hosts qkT
                #        j-tiles 1,3 (heads 2,3), c3 none
                #   hp1: c1 hosts proj(c0), c2 proj(c1), c3 proj(c2)
                # tail: proj(c3)
                gv = v_units()
                g13 = qk13_units()
                att_chunk(0, 0, gv, 1)
                att_chunk(0, 1, gv, 1)
                for _ in gv:
                    pass
                att_chunk(0, 2, g13, 1)
                for _ in g13:
                    pass
                att_chunk(0, 3, None, 1)
                att_chunk(1, 0, None, 1)
                gp = proj_units(0)
                att_chunk(1, 1, gp, 1)
                for _ in gp:
                    pass
                gp = proj_units(1)
                att_chunk(1, 2, gp, 1)
                for _ in gp:
                    pass
                gp = proj_units(2)
                att_chunk(1, 3, gp, 2)
                for _ in gp:
                    pass
                for _ in proj_units(3):
                    pass
    nc.compile()
    return nc


def _get_nc():
    if "nc" not in _cache:
        _cache["nc"] = _build()
    return _cache["nc"]


def kernel(x, W_attn, b_attn, W_proj, b_proj):
    x = np.asarray(x, dtype=np.float32)
    W_attn = np.asarray(W_attn, dtype=np.float32)
    b_attn = np.asarray(b_attn, dtype=np.float32)
    W_proj = np.asarray(W_proj, dtype=np.float32)
    b_proj = np.asarray(b_proj, dtype=np.float32)

    nc = _get_nc()
    in_maps = []
    for c in range(8):
        b, g = c // 4, c % 4
        in_maps.append({
            "xT": np.ascontiguousarray(x[b].T).astype(ml_dtypes.bfloat16),
            "w_qk": np.ascontiguousarray(
                np.concatenate([W_attn[:, g * V:(g + 1) * V],
                                W_attn[:, C + g * V:C + (g + 1) * V]], axis=1))
                .astype(ml_dtypes.bfloat16),
            "b_qk": np.ascontiguousarray(
                np.concatenate([b_attn[g * V:(g + 1) * V],
                                b_attn[C + g * V:C + (g + 1) * V]])
                .reshape(QK, 1)),
            "w_v": np.ascontiguousarray(W_attn[:, 2 * C + g * V:2 * C + (g + 1) * V])
                .astype(ml_dtypes.bfloat16),
            "b_v": np.ascontiguousarray(b_attn[2 * C + g * V:2 * C + (g + 1) * V]
                                        .reshape(1, V)),
            "w_pr": np.ascontiguousarray(W_proj[g * V:(g + 1) * V, :])
                .astype(ml_dtypes.bfloat16),
        })

    trace = os.environ.get("KTRACE") == "1"
    res = run_bass_kernel_spmd(nc, in_maps, core_ids=list(range(8)),
                               trace=trace)
    _cache["last_exec_ns"] = res.exec_time_ns
    _cache["last_result"] = res

    out = np.zeros((B, T, C), dtype=np.float32)
    for c in range(8):
        out[c // 4] += res.results[c]["out"].astype(np.float32)
    out += b_proj[None, None, :]
    return out
